# revision 37
# baseline (speedup 1.0000x reference)
"""Trainium2 Bass kernel for margin-ranking + weighted-BCE loss pair.

Math
----
Labels are binary {0,1}: same-label pairs each contribute relu(m) (a
count, N_eq), cross-label pairs contribute relu(c_a + p_b) with
c_a = m - p_pos, p_b = p_neg -- an outer sum.

Instead of materializing the ~18M-element outer sum, quantize each
positive's threshold t_a = p_a - m to a 128-level grid T (one level per
SBUF partition) and use CDF aggregates of the negatives:

  sum_b relu(c_a + p_b) = c_a*K(t_a) + S(t_a),
  K(t) = #{p_b > t},  S(t) = sum_{p_b > t} p_b

at the nearest grid level (second-order quantization error, measured
3e-5 relative). The aggregates are ADDITIVE over value shards and the
host combine is separable in (positives, negatives), so the 8 cores
just split all 8448 padded values evenly -- each scans its 512
negatives + 544 positives with FOUR fused threshold-scan instructions
(grid level on the partition axis as per-partition scalar/bias):

  K  = sum_b (p_b > T_l)            DVE tensor_scalar is_gt + accum
  R  = sum_b relu(p_b - T_l)        ACT activation Relu bias=-T + accum
  B  = sum_a (p_a > E_l + m)        DVE (E = inter-level edges)
  Rp = sum_a relu(p_a - E_l - m)    ACT

The host sums the per-core [128]-vectors, forms S = R + T*K and
A = -Rp - E*B, and telescopes
  total = K_0*C + Na*S_0 + sum_l dK_l*A_{l-1} + dS_l*B_{l-1}.
Pads (+16/-16) land on a sentinel top level with K=S=0 and contribute
exactly 0. BCE uses a degree-4 polynomial softplus on GpSimd (3.3e-3
max err, ~50x inside the 2e-2 gate). No matmuls, no PSUM, PE idle;
per-core input is one ~270KB value tile + constants.
"""

import numpy as np
import ml_dtypes

import concourse.bacc as bacc
import concourse.bass as bass
import concourse.mybir as mybir
import concourse.tile as tile
from concourse.bass_utils import run_bass_kernel_spmd

B = 8192
NCORES = 8
NP = 4352                  # padded positive count
PVALS = NP // NCORES       # 544 positives scanned per core
PAD_POS = 16.0
PAD_NEG = -16.0
P = 128
NLEV = 128
BCE_N = B // NCORES
BCE_F = BCE_N // P         # 8

# pck cols: z(8) | t(8) | pw | Tg | mTg | E2g | mE2g
PCK = 2 * BCE_F + 5
OUTC = 5                   # K | R | B | Rp | bce  (per-level rows)

G_COEF = (0.0010178200381822816, -0.01991946418641522, 0.14845389331661793,
          -0.5088132101257081, 0.6934405933221748)

f32 = mybir.dt.float32
bf16 = mybir.dt.bfloat16


def _grids(margin: float):
    T = np.concatenate([np.linspace(-6.0, 4.5, NLEV - 1), [16.0]])
    E = (T[:-1] + T[1:]) / 2
    E2 = np.concatenate([E + margin, [1e4]])
    return T.astype(np.float64), E2.astype(np.float64)


def _build_program(margin: float, nvals: int):
    from contextlib import ExitStack

    nc = bacc.Bacc("TRN2", target_bir_lowering=False, debug=False,
                   num_devices=NCORES)
    Relu = mybir.ActivationFunctionType.Relu
    add = mybir.AluOpType.add
    mult = mybir.AluOpType.mult
    igt = mybir.AluOpType.is_gt
    W = nvals + PVALS

    vals_d = nc.dram_tensor("vals", [P, W], bf16, kind="ExternalInput")
    pck_d = nc.dram_tensor("pck", [P, PCK], f32, kind="ExternalInput")
    out_d = nc.dram_tensor("out", [P, OUTC], f32, kind="ExternalOutput")

    with tile.TileContext(nc) as tc, ExitStack() as ctx:
        small = ctx.enter_context(tc.tile_pool(name="small", bufs=1))

        vals = small.tile([P, W], bf16, tag="vals")
        pck = small.tile([P, PCK], f32, tag="pck")
        # negatives half on the scalar queue, constants + positives half
        # on the gpsimd queue: one leading DMA per engine-owned queue.
        nc.scalar.dma_start(out=vals[:, 0:nvals], in_=vals_d[:, 0:nvals])
        nc.gpsimd.dma_start(out=pck[:, :], in_=pck_d[:, :])
        nc.gpsimd.dma_start(out=vals[:, nvals:W], in_=vals_d[:, nvals:W])

        zt = pck[:, 0:BCE_F]
        tt = pck[:, BCE_F:2 * BCE_F]
        pwt = pck[:, 2 * BCE_F:2 * BCE_F + 1]
        tg = pck[:, 2 * BCE_F + 1:2 * BCE_F + 2]
        mtg = pck[:, 2 * BCE_F + 2:2 * BCE_F + 3]
        e2g = pck[:, 2 * BCE_F + 3:2 * BCE_F + 4]
        me2g = pck[:, 2 * BCE_F + 4:2 * BCE_F + 5]

        tiny = small.tile([1, 1], f32, tag="tiny")
        nc.vector.memset(tiny[:, :], 1.0)
        # pre-load ACT's relu table during the DMA window
        nc.scalar.activation(tiny[:, 0:1], tiny[:, 0:1], Relu)

        outv = small.tile([P, OUTC], f32, tag="outv")
        mz = small.tile([P, BCE_F], f32, tag="mz")
        az = small.tile([P, BCE_F], f32, tag="az")
        mv = small.tile([P, BCE_F], f32, tag="mv")
        nc.vector.tensor_scalar_mul(mz[:, :], pck[:, 0:BCE_F], -1.0)
        nc.vector.tensor_scalar_max(mv[:, :], mz[:, :], 0.0)
        nc.vector.tensor_scalar_max(az[:, :], pck[:, 0:BCE_F], 0.0)
        nc.vector.tensor_add(az[:, :], az[:, :], mv[:, :])
        scrk = small.tile([P, nvals], bf16, tag="scrk")
        scrr = small.tile([P, nvals], bf16, tag="scrr")
        scrb = small.tile([P, PVALS], bf16, tag="scrb")
        scrp = small.tile([P, PVALS], bf16, tag="scrp")

        # ---- the four fused threshold scans ------------------------------
        nc.vector.tensor_scalar(scrk[:, :], vals[:, 0:nvals], tg, 0.0,
                                igt, add, accum_out=outv[:, 0:1])
        nc.scalar.activation(scrr[:, :], vals[:, 0:nvals], Relu, bias=mtg,
                             accum_out=outv[:, 1:2])
        nc.vector.tensor_scalar(scrb[:, :], vals[:, nvals:W], e2g, 0.0,
                                igt, add, accum_out=outv[:, 2:3])
        nc.scalar.activation(scrp[:, :], vals[:, nvals:W], Relu, bias=me2g,
                             accum_out=outv[:, 3:4])

        # ---- BCE: (1-t)z + (1+(pw-1)t)*(relu(-z)+poly(|z|)) --------------
        # independent products on GpSimd in parallel; serial poly chain on
        # DVE after its scans (3x lower tiny-op dispatch than GpSimd).
        gp = small.tile([P, BCE_F], f32, tag="gp")
        gt_ = small.tile([P, BCE_F], f32, tag="gt_")
        sp = small.tile([P, BCE_F], f32, tag="sp")
        wv = small.tile([P, BCE_F], f32, tag="wv")
        tz = small.tile([P, BCE_F], f32, tag="tz")
        r2 = small.tile([P, BCE_F], f32, tag="r2")
        bel = small.tile([P, BCE_F], f32, tag="bel")
        pwm1 = small.tile([P, 1], f32, tag="pwm1")

        # poly chain on GpSimd in parallel with DVE's scans; the cheap
        # product ops ride DVE after its B scan; short DVE tail combines.
        nc.gpsimd.tensor_scalar(gp[:, :], az[:, :], G_COEF[0], G_COEF[1],
                                mult, add)
        for ci in G_COEF[2:]:
            nc.gpsimd.tensor_mul(gt_[:, :], gp[:, :], az[:, :])
            nc.gpsimd.tensor_scalar_add(gp[:, :], gt_[:, :], ci)
        g = nc.vector
        g.tensor_scalar_add(pwm1[:, :], pwt, -1.0)
        g.tensor_scalar(wv[:, :], tt, pwm1[:, 0:1], 1.0, mult, add)
        g.tensor_mul(tz[:, :], tt, zt)
        g.tensor_sub(r2[:, :], zt, tz[:, :])
        g.tensor_add(sp[:, :], gp[:, :], mv[:, :])
        g.tensor_mul(bel[:, :], wv[:, :], sp[:, :])
        g.tensor_add(bel[:, :], bel[:, :], r2[:, :])
        nc.vector.tensor_reduce(outv[:, 4:5], bel[:, :],
                                axis=mybir.AxisListType.X, op=add)

        # ship aggregate columns as they complete: K/R after the first
        # scans, B/Rp after the second, bce last -- only the final tiny
        # column pays its DMA flight on the critical path.
        nc.sync.dma_start(out=out_d[:, 0:2], in_=outv[:, 0:2])
        nc.sync.dma_start(out=out_d[:, 2:4], in_=outv[:, 2:4])
        nc.sync.dma_start(out=out_d[:, 4:5], in_=outv[:, 4:5])

    nc.compile()
    return nc


_programs: dict = {}


def _get_program(margin: float, nvals: int):
    key = (margin, nvals)
    if key not in _programs:
        _programs[key] = _build_program(margin, nvals)
    return _programs[key]


def _make_in_maps(preds, labels, logits, targets, pos_weight, margin):
    p = np.ascontiguousarray(np.asarray(preds, np.float32))
    l = np.ascontiguousarray(np.asarray(labels, np.float32))
    z = np.ascontiguousarray(np.asarray(logits, np.float32))
    tg_ = np.ascontiguousarray(np.asarray(targets, np.float32))
    pw = float(np.asarray(pos_weight, np.float32).reshape(-1)[0])
    ndt = ml_dtypes.bfloat16

    mask = l >= 0.5
    pos = p[mask]
    neg = p[~mask]
    n1, n0 = len(pos), len(neg)
    nn = 4096 if n0 <= 4096 else 4352
    nvals = nn // NCORES
    assert n1 <= NP and n0 <= nn, (n1, n0)
    posf = np.full(NP, PAD_POS, np.float32)
    posf[:n1] = pos
    negf = np.full(nn, PAD_NEG, np.float32)
    negf[:n0] = neg
    negb = negf.astype(ndt)
    posb = posf.astype(ndt)

    T, E2 = _grids(float(margin))
    in_maps = []
    for c in range(NCORES):
        ns = negb[c * nvals:(c + 1) * nvals]
        ps = posb[c * PVALS:(c + 1) * PVALS]
        vrow = np.concatenate([ns, ps])
        vals = np.ascontiguousarray(
            np.broadcast_to(vrow, (P, nvals + PVALS)))
        pck = np.empty((P, PCK), np.float32)
        pck[:, 0:BCE_F] = z[BCE_N * c: BCE_N * (c + 1)].reshape(P, BCE_F)
        pck[:, BCE_F:2 * BCE_F] = \
            tg_[BCE_N * c: BCE_N * (c + 1)].reshape(P, BCE_F)
        pck[:, 2 * BCE_F] = pw
        pck[:, 2 * BCE_F + 1] = T
        pck[:, 2 * BCE_F + 2] = -T
        pck[:, 2 * BCE_F + 3] = E2
        pck[:, 2 * BCE_F + 4] = -E2
        in_maps.append({"vals": vals, "pck": pck})
    return in_maps, n0, n1, posf, nvals


def _combine(outs, margin: float, n0: int, n1: int,
             posf: np.ndarray) -> np.ndarray:
    # outs: [NCORES, P, OUTC]; aggregates are additive over shards.
    m = float(margin)
    T, E2 = _grids(m)
    o = outs.astype(np.float64)
    K = o[:, :, 0].sum(axis=0)
    R = o[:, :, 1].sum(axis=0)
    Bv = o[:, :, 2].sum(axis=0)
    Rp = o[:, :, 3].sum(axis=0)
    S = R + T * K
    A = -Rp - (E2 - m) * Bv
    c_tot = (m - posf.astype(np.float64)).sum()
    tot = K[0] * c_tot + NP * S[0]
    tot += ((K[1:] - K[:-1]) * A[:-1]).sum()
    tot += ((S[1:] - S[:-1]) * Bv[:-1]).sum()
    s_bce = o[:, :, 4].sum()
    n_eq = 0.5 * (n0 * (n0 - 1) + n1 * (n1 - 1))
    margin_loss = (tot + n_eq * max(m, 0.0)) / B
    bce_loss = s_bce / B
    return np.array([margin_loss, bce_loss], dtype=np.float32)


def _run(inputs: dict, trace: bool = False, **spmd_kwargs):
    m = float(np.asarray(inputs["margin"]))
    in_maps, n0, n1, posf, nvals = _make_in_maps(
        inputs["preds"], inputs["labels"], inputs["logits"],
        inputs["targets"], inputs["pos_weight"], m)
    nc = _get_program(m, nvals)
    res = run_bass_kernel_spmd(nc, in_maps, core_ids=list(range(NCORES)),
                               trace=trace, **spmd_kwargs)
    outs = np.stack([np.asarray(r["out"], np.float32) for r in res.results])
    return _combine(outs, m, n0, n1, posf), res


def kernel(preds, labels, logits, targets, pos_weight, margin):
    out, _ = _run(dict(preds=preds, labels=labels, logits=logits,
                       targets=targets, pos_weight=pos_weight,
                       margin=margin))
    return out


# revision 38
# speedup vs baseline: 1.1126x; 1.1126x over previous
"""Trainium2 Bass kernel for margin-ranking + weighted-BCE loss pair.

Math
----
Labels are binary {0,1}: same-label pairs each contribute relu(m) (a
count, N_eq), cross-label pairs contribute relu(c_a + p_b) with
c_a = m - p_pos, p_b = p_neg -- an outer sum.

Instead of materializing the ~18M-element outer sum, quantize each
positive's threshold t_a = p_a - m to a 128-level grid T (one level per
SBUF partition) and use CDF aggregates of the negatives:

  sum_b relu(c_a + p_b) = c_a*K(t_a) + S(t_a),
  K(t) = #{p_b > t},  S(t) = sum_{p_b > t} p_b

at the nearest grid level (second-order quantization error, measured
3e-5 relative). The aggregates are ADDITIVE over value shards and the
host combine is separable in (positives, negatives), so the 8 cores
just split all 8448 padded values evenly -- each scans its 512
negatives + 544 positives with FOUR fused threshold-scan instructions
(grid level on the partition axis as per-partition scalar/bias):

  K  = sum_b (p_b > T_l)            DVE tensor_scalar is_gt + accum
  R  = sum_b relu(p_b - T_l)        ACT activation Relu bias=-T + accum
  B  = sum_a (p_a > E_l + m)        DVE (E = inter-level edges)
  Rp = sum_a relu(p_a - E_l - m)    ACT

The host sums the per-core [128]-vectors, forms S = R + T*K and
A = -Rp - E*B, and telescopes
  total = K_0*C + Na*S_0 + sum_l dK_l*A_{l-1} + dS_l*B_{l-1}.
Pads (+16/-16) land on a sentinel top level with K=S=0 and contribute
exactly 0. BCE uses a degree-4 polynomial softplus on GpSimd (3.3e-3
max err, ~50x inside the 2e-2 gate). No matmuls, no PSUM, PE idle;
per-core input is one ~270KB value tile + constants.
"""

import numpy as np
import ml_dtypes

import concourse.bacc as bacc
import concourse.bass as bass
import concourse.mybir as mybir
import concourse.tile as tile
from concourse.bass_utils import run_bass_kernel_spmd

B = 8192
NCORES = 8
NP = 4352                  # padded positive count
PVALS = NP // NCORES       # 544 positives scanned per core
PAD_POS = 16.0
PAD_NEG = -16.0
P = 128
NLEV = 128
BCE_N = B // NCORES
BCE_F = BCE_N // P         # 8

# pck cols: z(8) | t(8) | pw | Tg | mTg | E2g | mE2g
PCK = 2 * BCE_F + 5
OUTC = 5                   # K | R | B | Rp | bce  (per-level rows)

G_COEF = (0.0010178200381822816, -0.01991946418641522, 0.14845389331661793,
          -0.5088132101257081, 0.6934405933221748)

f32 = mybir.dt.float32
bf16 = mybir.dt.bfloat16


def _grids(margin: float):
    T = np.concatenate([np.linspace(-6.0, 4.5, NLEV - 1), [16.0]])
    E = (T[:-1] + T[1:]) / 2
    E2 = np.concatenate([E + margin, [1e4]])
    return T.astype(np.float64), E2.astype(np.float64)


def _build_program(margin: float, nvals: int):
    from contextlib import ExitStack

    nc = bacc.Bacc("TRN2", target_bir_lowering=False, debug=False,
                   num_devices=NCORES)
    Relu = mybir.ActivationFunctionType.Relu
    add = mybir.AluOpType.add
    mult = mybir.AluOpType.mult
    igt = mybir.AluOpType.is_gt
    W = nvals + PVALS

    vals_d = nc.dram_tensor("vals", [P, W], bf16, kind="ExternalInput")
    pck_d = nc.dram_tensor("pck", [P, PCK], f32, kind="ExternalInput")
    out_d = nc.dram_tensor("out", [P, OUTC], f32, kind="ExternalOutput")

    with tile.TileContext(nc) as tc, ExitStack() as ctx:
        small = ctx.enter_context(tc.tile_pool(name="small", bufs=1))

        vals = small.tile([P, W], bf16, tag="vals")
        pck = small.tile([P, PCK], f32, tag="pck")
        # negatives half on the scalar queue, constants + positives half
        # on the gpsimd queue: one leading DMA per engine-owned queue.
        nc.scalar.dma_start(out=vals[:, 0:nvals], in_=vals_d[:, 0:nvals])
        nc.gpsimd.dma_start(out=pck[:, :], in_=pck_d[:, :])
        nc.gpsimd.dma_start(out=vals[:, nvals:W], in_=vals_d[:, nvals:W])

        zt = pck[:, 0:BCE_F]
        tt = pck[:, BCE_F:2 * BCE_F]
        pwt = pck[:, 2 * BCE_F:2 * BCE_F + 1]
        tg = pck[:, 2 * BCE_F + 1:2 * BCE_F + 2]
        mtg = pck[:, 2 * BCE_F + 2:2 * BCE_F + 3]
        e2g = pck[:, 2 * BCE_F + 3:2 * BCE_F + 4]
        me2g = pck[:, 2 * BCE_F + 4:2 * BCE_F + 5]

        tiny = small.tile([1, 1], f32, tag="tiny")
        nc.vector.memset(tiny[:, :], 1.0)
        # pre-load ACT's relu table during the DMA window
        nc.scalar.activation(tiny[:, 0:1], tiny[:, 0:1], Relu)

        outv = small.tile([P, OUTC], f32, tag="outv")
        mz = small.tile([P, BCE_F], f32, tag="mz")
        az = small.tile([P, BCE_F], f32, tag="az")
        mv = small.tile([P, BCE_F], f32, tag="mv")
        nc.vector.tensor_scalar_mul(mz[:, :], pck[:, 0:BCE_F], -1.0)
        nc.vector.tensor_scalar_max(mv[:, :], mz[:, :], 0.0)
        nc.vector.tensor_scalar_max(az[:, :], pck[:, 0:BCE_F], 0.0)
        nc.vector.tensor_add(az[:, :], az[:, :], mv[:, :])
        scrk = small.tile([P, nvals], bf16, tag="scrk")
        scrr = small.tile([P, nvals], bf16, tag="scrr")
        scrb = small.tile([P, PVALS], bf16, tag="scrb")
        scrp = small.tile([P, PVALS], bf16, tag="scrp")

        # ---- the four fused threshold scans ------------------------------
        nc.vector.tensor_scalar(scrk[:, :], vals[:, 0:nvals], tg, 0.0,
                                igt, add, accum_out=outv[:, 0:1])
        nc.scalar.activation(scrr[:, :], vals[:, 0:nvals], Relu, bias=mtg,
                             accum_out=outv[:, 1:2])
        nc.vector.tensor_scalar(scrb[:, :], vals[:, nvals:W], e2g, 0.0,
                                igt, add, accum_out=outv[:, 2:3])
        nc.scalar.activation(scrp[:, :], vals[:, nvals:W], Relu, bias=me2g,
                             accum_out=outv[:, 3:4])

        # ---- BCE: (1-t)z + (1+(pw-1)t)*(relu(-z)+poly(|z|)) --------------
        # independent products on GpSimd in parallel; serial poly chain on
        # DVE after its scans (3x lower tiny-op dispatch than GpSimd).
        gp = small.tile([P, BCE_F], f32, tag="gp")
        gt_ = small.tile([P, BCE_F], f32, tag="gt_")
        sp = small.tile([P, BCE_F], f32, tag="sp")
        wv = small.tile([P, BCE_F], f32, tag="wv")
        tz = small.tile([P, BCE_F], f32, tag="tz")
        r2 = small.tile([P, BCE_F], f32, tag="r2")
        bel = small.tile([P, BCE_F], f32, tag="bel")
        pwm1 = small.tile([P, 1], f32, tag="pwm1")

        # poly chain on GpSimd in parallel with DVE's scans; the cheap
        # product ops ride DVE after its B scan; short DVE tail combines.
        nc.gpsimd.tensor_scalar(gp[:, :], az[:, :], G_COEF[0], G_COEF[1],
                                mult, add)
        for ci in G_COEF[2:]:
            nc.gpsimd.tensor_mul(gt_[:, :], gp[:, :], az[:, :])
            nc.gpsimd.tensor_scalar_add(gp[:, :], gt_[:, :], ci)
        g = nc.vector
        g.tensor_scalar_add(pwm1[:, :], pwt, -1.0)
        g.tensor_scalar(wv[:, :], tt, pwm1[:, 0:1], 1.0, mult, add)
        g.tensor_mul(tz[:, :], tt, zt)
        g.tensor_sub(r2[:, :], zt, tz[:, :])
        g.tensor_add(sp[:, :], gp[:, :], mv[:, :])
        g.tensor_mul(bel[:, :], wv[:, :], sp[:, :])
        g.tensor_add(bel[:, :], bel[:, :], r2[:, :])
        nc.vector.tensor_reduce(outv[:, 4:5], bel[:, :],
                                axis=mybir.AxisListType.X, op=add)

        nc.sync.dma_start(out=out_d[:, :], in_=outv[:, :])

    nc.compile()
    return nc


_programs: dict = {}


def _get_program(margin: float, nvals: int):
    key = (margin, nvals)
    if key not in _programs:
        _programs[key] = _build_program(margin, nvals)
    return _programs[key]


def _make_in_maps(preds, labels, logits, targets, pos_weight, margin):
    p = np.ascontiguousarray(np.asarray(preds, np.float32))
    l = np.ascontiguousarray(np.asarray(labels, np.float32))
    z = np.ascontiguousarray(np.asarray(logits, np.float32))
    tg_ = np.ascontiguousarray(np.asarray(targets, np.float32))
    pw = float(np.asarray(pos_weight, np.float32).reshape(-1)[0])
    ndt = ml_dtypes.bfloat16

    mask = l >= 0.5
    pos = p[mask]
    neg = p[~mask]
    n1, n0 = len(pos), len(neg)
    nn = 4096 if n0 <= 4096 else 4352
    nvals = nn // NCORES
    assert n1 <= NP and n0 <= nn, (n1, n0)
    posf = np.full(NP, PAD_POS, np.float32)
    posf[:n1] = pos
    negf = np.full(nn, PAD_NEG, np.float32)
    negf[:n0] = neg
    negb = negf.astype(ndt)
    posb = posf.astype(ndt)

    T, E2 = _grids(float(margin))
    in_maps = []
    for c in range(NCORES):
        ns = negb[c * nvals:(c + 1) * nvals]
        ps = posb[c * PVALS:(c + 1) * PVALS]
        vrow = np.concatenate([ns, ps])
        vals = np.ascontiguousarray(
            np.broadcast_to(vrow, (P, nvals + PVALS)))
        pck = np.empty((P, PCK), np.float32)
        pck[:, 0:BCE_F] = z[BCE_N * c: BCE_N * (c + 1)].reshape(P, BCE_F)
        pck[:, BCE_F:2 * BCE_F] = \
            tg_[BCE_N * c: BCE_N * (c + 1)].reshape(P, BCE_F)
        pck[:, 2 * BCE_F] = pw
        pck[:, 2 * BCE_F + 1] = T
        pck[:, 2 * BCE_F + 2] = -T
        pck[:, 2 * BCE_F + 3] = E2
        pck[:, 2 * BCE_F + 4] = -E2
        in_maps.append({"vals": vals, "pck": pck})
    return in_maps, n0, n1, posf, nvals


def _combine(outs, margin: float, n0: int, n1: int,
             posf: np.ndarray) -> np.ndarray:
    # outs: [NCORES, P, OUTC]; aggregates are additive over shards.
    m = float(margin)
    T, E2 = _grids(m)
    o = outs.astype(np.float64)
    K = o[:, :, 0].sum(axis=0)
    R = o[:, :, 1].sum(axis=0)
    Bv = o[:, :, 2].sum(axis=0)
    Rp = o[:, :, 3].sum(axis=0)
    S = R + T * K
    A = -Rp - (E2 - m) * Bv
    c_tot = (m - posf.astype(np.float64)).sum()
    tot = K[0] * c_tot + NP * S[0]
    tot += ((K[1:] - K[:-1]) * A[:-1]).sum()
    tot += ((S[1:] - S[:-1]) * Bv[:-1]).sum()
    s_bce = o[:, :, 4].sum()
    n_eq = 0.5 * (n0 * (n0 - 1) + n1 * (n1 - 1))
    margin_loss = (tot + n_eq * max(m, 0.0)) / B
    bce_loss = s_bce / B
    return np.array([margin_loss, bce_loss], dtype=np.float32)


def _run(inputs: dict, trace: bool = False, **spmd_kwargs):
    m = float(np.asarray(inputs["margin"]))
    in_maps, n0, n1, posf, nvals = _make_in_maps(
        inputs["preds"], inputs["labels"], inputs["logits"],
        inputs["targets"], inputs["pos_weight"], m)
    nc = _get_program(m, nvals)
    res = run_bass_kernel_spmd(nc, in_maps, core_ids=list(range(NCORES)),
                               trace=trace, **spmd_kwargs)
    outs = np.stack([np.asarray(r["out"], np.float32) for r in res.results])
    return _combine(outs, m, n0, n1, posf), res


def kernel(preds, labels, logits, targets, pos_weight, margin):
    out, _ = _run(dict(preds=preds, labels=labels, logits=logits,
                       targets=targets, pos_weight=pos_weight,
                       margin=margin))
    return out
